# revision 19
# baseline (speedup 1.0000x reference)
"""ASSD (average symmetric surface distance) kernel for Trainium2, 8 NeuronCores.

Problem: real_pts [16384,3], pred_pts [16384,3] in [0,128)^3.
  assd = (sum_i NNdist(pred_i, real) + sum_j NNdist(real_j, pred)) / 32768

Strategy (v2: block-diagonal quads + host fixup)
------------------------------------------------
Host: bin each query set into 8x8-unit (x,y) cells, z-sort inside each
cell, cut into SUB-BLOCKS of 32 queries. Each sub-block gets a candidate
window: reference points within MARGIN of its bounding box. MARGIN is
deliberately SMALL (~0.6, far below the max NN distance): any query
whose windowed min distance exceeds MARGIN-0.01 is recomputed exactly
on the host afterwards (a few % of queries). This shrinks windows to
~64-112 points (vs ~300 with a conservative margin).

HW: four sub-blocks are stacked into one "quad": a [108, 128]
block-diagonal bf16 weight matrix (band i = partitions 27i..27i+27
holds sub-block i's 27 augmented query rows in columns 32i..32i+32,
zeros elsewhere) multiplied against a [108, w] moving matrix whose band
i rows carry sub-block i's candidate window. One matmul → PSUM [128, w]
where lane m holds u[m, c] = r2 - 2 q_m.c over lane m's OWN sub-block
window; one DVE reduce_min per PSUM group of G quads. The off-diagonal
zeros guarantee no cross-band contamination. This runs the PE with
N=w~64-112 streams (vs 300) and cuts the DVE reduce (the previous
bottleneck) by ~3x.

The 27-row augmentation (bf16 splitting of coordinates and squared
coordinates) is unchanged from v1 and preserves fp32-grade accuracy at
full bf16 PE rate. The host adds ||q||^2 (commutes with min), takes
sqrt, applies the guard, fixes failures exactly, and sums. q2/r2 are
quantized to a calibrated grid to emulate the fp32 reference's rounding
noise. Exact brute-force fallback covers any pathological input.
"""

import numpy as np
import ml_dtypes

BF16 = ml_dtypes.bfloat16

SUB = 64           # queries per sub-block (one partition band)
BANDS = 2          # sub-blocks per quad
BLK = SUB * BANDS  # queries per quad (PE output partitions)
KROWS = 27         # augmented contraction rows per band
KTOT = KROWS * BANDS  # 54 contraction rows of the block-diagonal matmul
CX = 8             # x cells (16-unit)
CY = 8             # y cells
W_CAP = 192        # hard window cap (must be <= W_SLOT)
W_SLOT = 256       # PSUM fp32 slot per quad (2 quads per 512-fp32 bank:
                   # a matmul output must stay inside one PSUM bank)
G = 8              # quads per PSUM tile / per DVE reduce (4 banks)
MARGINS = (0.45, 0.4, 0.35, 0.3, 0.25)  # ladder: first feasible wins
N_CORES = 8
NOISE_A = 2.5      # fp32-reference rounding-noise emulation scale
BIG = 1.0e9        # pad candidate row value -> never the min
MAX_FIX = 20000    # above this many guard failures, just brute-force

_nc_cache = {}
LAST_RESULT = None  # BassKernelResults of the last HW run (for profiling)


def _build_bass(nb, ws, gsz, wchunks, cchunks):
    """Bass kernel: nb quads; group g has gsz[g] quads of window width
    ws[g]. Per quad: one [108,128]x[108,w] bf16 matmul into its PSUM
    slot; per group: one fp32 reduce_min -> acc column block.

    Input streaming uses both HW DGE pipelines: the weight image rides
    the ACT queue, the candidate image the sync queue, each in a few
    big-row chunks (per-partition descriptor runs of several KB), in
    demand order. wchunks: quad boundaries of the weight DMAs;
    cchunks: group boundaries of the candidate DMAs."""
    from concourse import mybir, tile, bacc

    f32 = mybir.dt.float32
    b16 = mybir.dt.bfloat16
    ng = len(ws)
    assert sum(gsz) == nb
    coff = [0]
    for w, sz in zip(ws, gsz):
        coff.append(coff[-1] + sz * w)
    ncc = coff[-1]

    nc = bacc.Bacc(enable_partition_id=False)
    wq_d = nc.declare_dram_parameter("wq", [KTOT, nb * BLK], b16,
                                     isOutput=False)
    cd_d = nc.declare_dram_parameter("cd", [KTOT, ncc], b16, isOutput=False)
    o_d = nc.declare_dram_parameter("o", [BLK, nb], f32, isOutput=True)

    with tile.TileContext(nc) as tc:
        with (
            tc.tile_pool(name="wp", bufs=1) as wp,
            tc.tile_pool(name="cp", bufs=1) as cp,
            tc.tile_pool(name="ps", bufs=2, space="PSUM") as pp,
            tc.tile_pool(name="ap", bufs=1) as apool,
        ):
            wimg = wp.tile([KTOT, nb * BLK], b16)
            cdt = cp.tile([KTOT, ncc], b16)
            acc = apool.tile([BLK, nb], f32)
            # Streams run 60-95 GB/s per queue but each DMA pays ~0.7us
            # generation and ~1.5us completion-semaphore latency. Balance
            # bytes over the three queues (sync-HW, scalar-HW, gpsimd-SW)
            # in demand order: group-0's gate rides the earliest-free
            # queues in small chunks; weight tail halves follow the
            # candidate chunks on the HW queues.
            HK = KTOT // 2
            for i in range(len(cchunks) - 1):
                g0, g1 = cchunks[i], cchunks[i + 1]
                nc.sync.dma_start(cdt[:HK, coff[g0]:coff[g1]],
                                  cd_d[:HK, coff[g0]:coff[g1]])
                nc.scalar.dma_start(cdt[HK:, coff[g0]:coff[g1]],
                                    cd_d[HK:, coff[g0]:coff[g1]])
            for q0, q1 in zip(wchunks[:-1], wchunks[1:]):
                nc.gpsimd.dma_start(wimg[:, q0 * BLK:q1 * BLK],
                                    wq_d[:, q0 * BLK:q1 * BLK])
            wtail = wchunks[-1]
            if wtail < nb:
                nc.sync.dma_start(wimg[HK:, wtail * BLK:],
                                  wq_d[HK:, wtail * BLK:])
                nc.scalar.dma_start(wimg[:HK, wtail * BLK:],
                                    wq_d[:HK, wtail * BLK:])
            slot = 0
            for g in range(ng):
                w, sz = ws[g], gsz[g]
                ps = pp.tile([BLK, G, W_SLOT], f32, tag="ps")
                for j in range(sz):
                    q = slot + j
                    nc.tensor.matmul(
                        ps[:, j, :w],
                        wimg[:, q * BLK:(q + 1) * BLK],
                        cdt[:, coff[g] + j * w:coff[g] + (j + 1) * w],
                    )
                nc.vector.tensor_reduce(
                    acc[:, slot:slot + sz], ps[:, :sz, :w],
                    axis=mybir.AxisListType.X, op=mybir.AluOpType.min,
                )
                slot += sz
            # output: row-split 3 ways so each queue carries ~43 descriptors
            nc.sync.dma_start(o_d[:43, :], acc[:43, :])
            nc.scalar.dma_start(o_d[43:86, :], acc[43:86, :])
            nc.gpsimd.dma_start(o_d[86:, :], acc[86:, :])
    nc.compile()
    return nc


def _ulp32(x):
    x = np.maximum(np.abs(x), 1e-30)
    return 2.0 ** (np.floor(np.log2(x)) - 23)


def _quant(vals, mags):
    """Quantize vals (fp64) to the NOISE_A*ulp32(mags) grid."""
    g = NOISE_A * _ulp32(mags)
    return np.round(vals / g) * g


def _split3(v):
    """fp64 array -> 3 bf16 pieces (as fp64 arrays) summing to ~v."""
    h = v.astype(BF16).astype(np.float64)
    l = (v - h).astype(BF16).astype(np.float64)
    m = (v - h - l).astype(BF16).astype(np.float64)
    return h, l, m


def _aug_rows(pts, eps0, is_query):
    """Build the [27, N] augmented row matrix (bf16) for a point set."""
    n = pts.shape[0]
    out = np.zeros((KROWS, n), BF16)
    ones = np.ones(n, BF16)
    for d in range(3):
        pd = pts[:, d].astype(np.float64)
        h, l, m = _split3(pd)
        base = 9 * d
        if is_query:
            q_h = (-2.0 * h).astype(BF16)
            q_l = (-2.0 * l).astype(BF16)
            q_m = (-2.0 * m).astype(BF16)
            out[base + 0] = ones
            out[base + 1] = q_h
            out[base + 2] = ones
            out[base + 3] = q_h
            out[base + 4] = q_l
            out[base + 5] = ones
            out[base + 6] = q_l
            out[base + 7] = q_h
            out[base + 8] = q_m
        else:
            s = pd * pd + (eps0 if d == 0 else 0.0)
            sh, sl, sm = _split3(s)
            out[base + 0] = sh.astype(BF16)
            out[base + 1] = h.astype(BF16)
            out[base + 2] = sl.astype(BF16)
            out[base + 3] = l.astype(BF16)
            out[base + 4] = h.astype(BF16)
            out[base + 5] = sm.astype(BF16)
            out[base + 6] = l.astype(BF16)
            out[base + 7] = m.astype(BF16)
            out[base + 8] = h.astype(BF16)
    return out


def _make_subblocks(qpts, rpts):
    """Cut queries into cell/z-sorted sub-blocks of <=SUB; gather each
    sub-block's candidate window with the largest feasible small margin.

    Returns (list of (q_ids, cand_ids) with ids into qpts/rpts, margin,
    ok)."""
    n = qpts.shape[0]
    xbin = np.minimum(qpts[:, 0] // (128.0 / CX), CX - 1).astype(np.int64)
    ybin = np.minimum(qpts[:, 1] // (128.0 / CY), CY - 1).astype(np.int64)
    cell = xbin * CY + ybin
    order = np.lexsort((qpts[:, 2], cell))
    qs = qpts[order]
    ss = cell[order]

    rz = rpts[:, 2]
    rorder = np.argsort(rz)
    rz_s = rz[rorder]
    rx_s = rpts[rorder, 0]
    ry_s = rpts[rorder, 1]

    bounds = []
    start = 0
    while start < n:
        send = np.searchsorted(ss, ss[start], side="right")
        bend = min(start + SUB, send)
        mem = qs[start:bend]
        bounds.append((start, bend,
                       mem[:, 0].min(), mem[:, 0].max(),
                       mem[:, 1].min(), mem[:, 1].max(),
                       mem[:, 2].min(), mem[:, 2].max()))
        start = bend

    def windows(margin):
        res = []
        for (s0, s1, xlo, xhi, ylo, yhi, zlo, zhi) in bounds:
            i0 = np.searchsorted(rz_s, zlo - margin, side="left")
            i1 = np.searchsorted(rz_s, zhi + margin, side="right")
            keep = ((rx_s[i0:i1] >= xlo - margin) & (rx_s[i0:i1] <= xhi + margin)
                    & (ry_s[i0:i1] >= ylo - margin) & (ry_s[i0:i1] <= yhi + margin))
            if keep.sum() > W_CAP:
                return None
            res.append(rorder[i0:i1][keep])
        return res

    for margin in MARGINS:
        wins = windows(margin)
        if wins is not None:
            sbs = [(order[s0:s1], cand)
                   for (s0, s1, *rest), cand in zip(bounds, wins)]
            return sbs, margin, True
    return None, 0.0, False


def _brute_force(real, pred):
    """Exact fallback, mirrors reference numerics in fp32 (blocked)."""
    def nn_sum(q, r):
        r2 = (r * r).sum(1, dtype=np.float32)[None, :]
        q2 = (q * q).sum(1, dtype=np.float32)[:, None]
        tot = 0.0
        for i in range(0, q.shape[0], 1024):
            d2 = q2[i:i + 1024] + r2 - np.float32(2.0) * (q[i:i + 1024] @ r.T)
            d2 = np.maximum(d2, 0.0)
            tot += np.sqrt(d2.min(1)).astype(np.float64).sum()
        return tot
    n = real.shape[0] + pred.shape[0]
    return (nn_sum(pred, real) + nn_sum(real, pred)) / n


def _exact_nn(q, r):
    """Exact fp64 NN distances of queries q against full set r."""
    out = np.empty(q.shape[0])
    r = r.astype(np.float64)
    r2 = (r * r).sum(1)
    for i in range(0, q.shape[0], 512):
        qi = q[i:i + 512].astype(np.float64)
        d2 = (qi * qi).sum(1)[:, None] + r2[None, :] - 2.0 * (qi @ r.T)
        out[i:i + 512] = np.sqrt(np.maximum(d2.min(1), 0.0))
    return out


def kernel(real_pts, pred_pts):
    global LAST_RESULT
    real = np.ascontiguousarray(np.asarray(real_pts, dtype=np.float32))
    pred = np.ascontiguousarray(np.asarray(pred_pts, dtype=np.float32))

    if (real.shape[0] < 1024 or pred.shape[0] < 1024
            or not np.isfinite(real).all() or not np.isfinite(pred).all()):
        return np.float32(_brute_force(real, pred))

    sb1, mg1, ok1 = _make_subblocks(pred, real)   # pred -> real
    sb2, mg2, ok2 = _make_subblocks(real, pred)   # real -> pred
    if not (ok1 and ok2):
        return np.float32(_brute_force(real, pred))

    # augmented matrices + noise-emulated squared norms per direction
    r2a = (real.astype(np.float64) ** 2).sum(1)
    p2a = (pred.astype(np.float64) ** 2).sum(1)
    R1 = _aug_rows(real, _quant(r2a, 2 * r2a) - r2a, is_query=False)
    R2 = _aug_rows(pred, _quant(p2a, 2 * p2a) - p2a, is_query=False)
    Q1 = _aug_rows(pred, None, is_query=True)
    Q2 = _aug_rows(real, None, is_query=True)
    q2n1 = _quant(p2a, 2 * p2a)
    q2n2 = _quant(r2a, 2 * r2a)

    # unified sub-block list: (Q, R, q2n, qids, global_qids, cids, guard)
    # global query ids: pred queries at [0, npred), real at [npred, ntot)
    npred = pred.shape[0]
    subs = ([(Q1, R1, q2n1, q, q, c, mg1 - 0.01) for q, c in sb1]
            + [(Q2, R2, q2n2, q, q + npred, c, mg2 - 0.01) for q, c in sb2])
    counts = np.array([len(c) for (_, _, _, _, _, c, _) in subs])

    # sort sub-blocks by count; consecutive 4 -> quad; deal quads
    # round-robin so all cores share the same width profile per slot.
    rank = np.argsort(counts, kind="stable")
    nsb = len(subs)
    nq = -(-nsb // BANDS)                 # quads (global)
    nb = -(-nq // N_CORES)                # quads per core
    nqp = nb * N_CORES                    # padded global quads
    # sub-block rank r -> quad r // BANDS, band r % BANDS
    # quad rank p -> core p % N_CORES, slot p // N_CORES

    gsz = []
    left = nb
    while left > 0:
        gsz.append(min(G, left))
        left -= gsz[-1]
    ng = len(gsz)
    ws = []
    s0 = 0
    for sz in gsz:
        hi = min((s0 + sz) * N_CORES * BANDS, nsb)
        lo = s0 * N_CORES * BANDS
        wg = int(counts[rank[lo:hi]].max()) if hi > lo else 1
        ws.append(max(16, min(W_CAP, -(-wg // 16) * 16)))
        s0 += sz
    ws = tuple(ws)
    gsz = tuple(gsz)

    # weight-image DMA split: head quads on SWDGE (2 chunks), tail halves
    # ride the HW queues behind the candidate chunks (byte balancing)
    wtail = min(max(nb * 20 // 33, G), nb)
    wchunks = tuple(sorted(set([0, min(G, wtail), wtail])))
    # candidate chunks by group: g0 alone (the startup gate), then halves
    if ng >= 3:
        cchunks = (0, 1, (1 + ng) // 2, ng)
    else:
        cchunks = tuple(range(ng + 1))

    key = (nb, ws, gsz, wchunks, cchunks)
    if key not in _nc_cache:
        _nc_cache.clear()
        _nc_cache[key] = _build_bass(nb, ws, gsz, wchunks, cchunks)
    nc = _nc_cache[key]

    coff = [0]
    for w, sz in zip(ws, gsz):
        coff.append(coff[-1] + sz * w)
    ncc = coff[-1]

    wq = np.zeros((N_CORES, KTOT, nb * BLK), BF16)
    cd = np.zeros((N_CORES, KTOT, ncc), BF16)
    q2b = np.zeros((N_CORES, nb, BLK))
    guards = np.full((N_CORES, nb, BLK), 1e9)
    qidm = np.full((N_CORES, nb, BLK), -1, np.int64)

    slot_of = np.empty(ng + 1, np.int64)
    slot_of[0] = 0
    for g in range(ng):
        slot_of[g + 1] = slot_of[g] + gsz[g]
    # per (group, in-group slot j, core, band): fill images
    for g in range(ng):
        w = ws[g]
        for j in range(gsz[g]):
            slot = slot_of[g] + j
            for core in range(N_CORES):
                p = slot * N_CORES + core     # global quad rank
                for band in range(BANDS):
                    r = p * BANDS + band      # sub-block rank index
                    if r >= nsb:
                        continue
                    Q, R, q2n, qids, gids, cids, guard = subs[rank[r]]
                    cnt = len(qids)
                    wc = len(cids)
                    rowa = KROWS * band
                    cola = slot * BLK + SUB * band
                    wq[core, rowa:rowa + KROWS, cola:cola + cnt] = Q[:, qids]
                    ccol = coff[g] + j * w
                    cd[core, rowa:rowa + KROWS, ccol:ccol + wc] = R[:, cids]
                    cd[core, rowa, ccol + wc:ccol + w] = BF16(BIG)
                    q2b[core, slot, SUB * band:SUB * band + cnt] = q2n[qids]
                    guards[core, slot, SUB * band:SUB * band + cnt] = guard
                    qidm[core, slot, SUB * band:SUB * band + cnt] = gids
    del subs

    from concourse.bass_utils import run_bass_kernel_spmd
    in_maps = [{"wq": np.ascontiguousarray(wq[i]),
                "cd": np.ascontiguousarray(cd[i])} for i in range(N_CORES)]
    res = run_bass_kernel_spmd(nc, in_maps, list(range(N_CORES)))
    LAST_RESULT = res

    ntot = real.shape[0] + npred
    d_all = np.empty(ntot)
    d_all.fill(np.nan)
    nfail = 0
    fail_q = []          # global query ids failing the guard
    for core in range(N_CORES):
        o = res.results[core]["o"]        # [BLK, nb]
        u = o.T.astype(np.float64)        # [nb, BLK]
        d2 = q2b[core] + u
        d = np.sqrt(np.maximum(d2, 0.0))
        valid = qidm[core] >= 0
        ok = valid & (d <= guards[core])
        bad = valid & ~ok
        gid = qidm[core]
        d_all[gid[ok]] = d[ok]
        fail_q.append(gid[bad])
        nfail += int(bad.sum())
    if nfail > MAX_FIX:
        return np.float32(_brute_force(real, pred))
    if nfail:
        ids = np.concatenate(fail_q)
        is2 = ids >= npred
        ids1 = ids[~is2]
        ids2 = ids[is2] - npred
        if ids1.size:
            d_all[ids1] = _exact_nn(pred[ids1], real)
        if ids2.size:
            d_all[ids2 + npred] = _exact_nn(real[ids2], pred)
    if np.isnan(d_all).any():
        return np.float32(_brute_force(real, pred))
    assd = d_all.sum() / ntot
    return np.float32(assd)
